# revision 13
# baseline (speedup 1.0000x reference)
"""Bass/Trainium2 kernel for 3-layer GAT over 8 NeuronCores.

Sharding: core 2b+h handles (batch b, dst-half h). v2 design: all per-node
tables live in SBUF as one bf16 tensor T[128 chan, NE, 2] (d=2 packed, plane
j = src half). Per-edge data is fetched with gpsimd ap_gather (SBUF-local, no
SWDGE DMA descriptors - the v1 bottleneck), then rotated into row-major
per-edge layout with one batched DMA xbar transpose per (side, supertile).

Table channel layout, per 16-channel block b (block = one gpsimd core):
  chan 16b+u, u<8 : h feature 8b+u       (head = b//2 for H=4)
  chan 16b+8      : esrc value b (b<H)
  chan 16b+12+w   : edst value w, replicated in every block so the
                    pair-bucketed dst gather (per-core idx) sees it
Plane j holds half-j values; a per-core mask input selects the own half's
edst rows (copy_predicated), keeping the program SPMD-identical.

Edges are packed on the host exactly as v1: dst-sorted, paired bins of 128
slots, <=15 segments/pair (slot 15 dummy), inflated ids 16*pair+slot for
layers 2/3; aggregation stays the proven one-hot PSUM matmul per pair, and
normalization/bias/relu feed xn [64, infl] which AllGathers between the two
half-cores of a batch.
"""

import numpy as np

import concourse.bass as bass
import concourse.tile as tile
from concourse import bacc, mybir
from concourse.bass_utils import run_bass_kernel_spmd

F32 = mybir.dt.float32
BF16 = mybir.dt.bfloat16
I16 = mybir.dt.int16

NEG_SLOPE = 0.2
EDGE_DEPTH = 0
EPS = 1e-16
P = 128
NSEG_MAX = 15          # segments per pair (slot 15 reserved for dummies)
PAIRS_PER_BATCH = 32   # 4 supertiles of 8 pairs

# Problem dims (hardcoded per the task contract)
N_NODES = 50000
B = 4
F_IN = 128
H, C = 4, 16
HC = H * C            # 64
N_CLS = 16


# ----------------------------------------------------------------------------
# Host preprocessing (edge packing identical to v1)
# ----------------------------------------------------------------------------

def _pack_half(src, dst, n_lo, n_hi, half):
    half_n = n_hi - n_lo
    sel = (dst >= n_lo) & (dst < n_hi)
    s_, d_ = src[sel], dst[sel]
    order = np.argsort(d_, kind="stable")
    s_, d_ = s_[order], d_[order]
    uniq, seg_start = np.unique(d_, return_index=True)
    assert len(uniq) == half_n, "self-loops guarantee every node is a dst"
    seg_len = np.diff(np.append(seg_start, len(d_)))
    a_side = s_ < N_HALF_GLOBAL[0]  # bin A: src in global half 0
    pairs = []
    cur, curA, curB = [], 0, 0
    for i in range(half_n):
        a0, L = seg_start[i], seg_len[i]
        la = int(a_side[a0 : a0 + L].sum())
        lb = int(L - la)
        if len(cur) >= NSEG_MAX or curA + la > P or curB + lb > P:
            pairs.append(cur)
            cur, curA, curB = [], 0, 0
        cur.append(i)
        curA += la
        curB += lb
    if cur:
        pairs.append(cur)
    np_real = len(pairs)
    out = dict(np_real=np_real)
    npad = -(-np_real // PAIRS_PER_BATCH) * PAIRS_PER_BATCH
    srcA = np.zeros((npad, P), np.int64)
    srcB = np.full((npad, P), N_HALF_GLOBAL[0], np.int64)
    dstv = np.full((npad, 2, P), n_lo, np.int64)
    slotA = np.full((npad, P), NSEG_MAX, np.float32)
    slotB = np.full((npad, P), NSEG_MAX, np.float32)
    seg_node = np.full((npad, 16), -1, np.int64)
    for k, segs in enumerate(pairs):
        ea = eb = 0
        for s_i, seg in enumerate(segs):
            a0, L = seg_start[seg], seg_len[seg]
            e_src = s_[a0 : a0 + L]
            e_a = e_src[a_side[a0 : a0 + L]]
            e_b = e_src[~a_side[a0 : a0 + L]]
            la, lb = len(e_a), len(e_b)
            srcA[k, ea : ea + la] = e_a
            slotA[k, ea : ea + la] = s_i
            dstv[k, 0, ea : ea + la] = uniq[seg]
            srcB[k, eb : eb + lb] = e_b
            slotB[k, eb : eb + lb] = s_i
            dstv[k, 1, eb : eb + lb] = uniq[seg]
            seg_node[k, s_i] = uniq[seg]
            ea += la
            eb += lb
    out.update(srcA=srcA, srcB=srcB, dstv=dstv, slotA=slotA, slotB=slotB,
               seg_node=seg_node, npad=npad)
    return out


N_HALF_GLOBAL = [None]


def preprocess(edge_index, n_nodes):
    src = np.asarray(edge_index[0], np.int64)
    dst = np.asarray(edge_index[1], np.int64)
    loop = np.arange(n_nodes, dtype=np.int64)
    src = np.concatenate([src, loop])
    dst = np.concatenate([dst, loop])
    half = n_nodes // 2
    N_HALF_GLOBAL[0] = half
    packs = [_pack_half(src, dst, 0, half, 0),
             _pack_half(src, dst, half, n_nodes, 1)]
    npairs = max(p["npad"] for p in packs)
    npairs = -(-npairs // PAIRS_PER_BATCH) * PAIRS_PER_BATCH
    infl = 16 * npairs
    assert infl <= 32768, f"inflated id space {infl} exceeds int16 range"
    node_pad = -(-half // P) * P
    assert node_pad <= 32768
    for h, pk in enumerate(packs):
        k = npairs - pk["npad"]
        if k:
            for name, fill in [("srcA", 0), ("srcB", half), ("dstv", h * half),
                               ("slotA", NSEG_MAX), ("slotB", NSEG_MAX),
                               ("seg_node", -1)]:
                arr = pk[name]
                pad_shape = (k,) + arr.shape[1:]
                pk[name] = np.concatenate(
                    [arr, np.full(pad_shape, fill, arr.dtype)])
        inv = np.full(half, -1, np.int64)
        sn = pk["seg_node"].reshape(-1)
        valid = sn >= 0
        inv[sn[valid] - h * half] = np.nonzero(valid)[0]
        assert (inv >= 0).all()
        pk["infl_of_node"] = inv
    return dict(packs=packs, npairs=npairs, infl=infl, half=half,
                node_pad=node_pad, n_batches=npairs // PAIRS_PER_BATCH)


def _wrap_idx(flat):
    """ap_gather idx layout: idx i at [i%16, i//16], replicated to 128
    partitions (all 8 gpsimd cores use the same list)."""
    n = len(flat)
    assert n % 16 == 0
    w = np.asarray(flat, np.int64).reshape(n // 16, 16).T
    assert w.max() < 32768 and w.min() >= -32768
    return np.tile(w.astype(np.int16), (8, 1))


def _wrap_idx_core(flat):
    """Per-core idx wrap: [16, n/16] for ONE gpsimd core."""
    n = len(flat)
    assert n % 16 == 0
    w = np.asarray(flat, np.int64).reshape(n // 16, 16).T
    assert w.max() < 32768 and w.min() >= 0
    return w.astype(np.int16)


def build_core_idx_arrays(pp, h):
    """Per-core (half h) gather idx/slot arrays for all batches."""
    pk = pp["packs"][h]
    half = pp["half"]
    nb = pp["n_batches"]
    E_BLK = PAIRS_PER_BATCH * P  # 4096
    srcA = pk["srcA"].reshape(nb, E_BLK)
    srcB = pk["srcB"].reshape(nb, E_BLK)
    dstv = pk["dstv"].reshape(nb, PAIRS_PER_BATCH, 2, P)
    inflS = [pp["packs"][0]["infl_of_node"], pp["packs"][1]["infl_of_node"]]

    def loc(ids, src_half):
        return ids - src_half * half

    def infl_map(ids, src_half):
        return inflS[src_half][ids - src_half * half]

    out = {}
    for tag, f in [("1", loc), ("2", infl_map)]:
        out[f"srcA{tag}"] = np.stack([_wrap_idx(f(srcA[i], 0)) for i in range(nb)])
        out[f"srcB{tag}"] = np.stack([_wrap_idx(f(srcB[i], 1)) for i in range(nb)])
        # pair-bucketed dst idx: core g holds pairs 8*st+g, order (st, side)
        dstk = np.zeros((nb, P, 64), np.int16)
        for k in range(nb):
            ids = f(dstv[k], h)  # [32, 2, 128] local/infl dst ids
            for g in range(8):
                flat = np.concatenate(
                    [ids[8 * st + g, s] for st in range(4) for s in range(2)])
                dstk[k, 16 * g : 16 * (g + 1)] = _wrap_idx_core(flat)
        out[f"dstP{tag}"] = dstk
    out["slotA"] = pk["slotA"].reshape(nb, PAIRS_PER_BATCH, P).transpose(0, 2, 1).copy()
    out["slotB"] = pk["slotB"].reshape(nb, PAIRS_PER_BATCH, P).transpose(0, 2, 1).copy()
    return out


def augment_weights(W, a_s, a_d):
    """[F, HC] weights -> [F, 2H + HC] table weights, cols [edst|esrc|h]."""
    Hh, Cc = a_s.shape
    W64 = np.asarray(W, np.float64)
    As = np.zeros((Hh * Cc, Hh))
    Ad = np.zeros((Hh * Cc, Hh))
    for hh in range(Hh):
        As[hh * Cc : (hh + 1) * Cc, hh] = np.asarray(a_s, np.float64)[hh]
        Ad[hh * Cc : (hh + 1) * Cc, hh] = np.asarray(a_d, np.float64)[hh]
    return np.concatenate([W64 @ Ad, W64 @ As, W64], axis=1).astype(np.float32)


def pack_weights(Wa, nh):
    """[F, 2nh+nf] augmented weights -> [F, 128] packed table weights.

    col 16b+u (u<8) = h feat 8b+u; col 16b+8 = esrc b (b<nh);
    col 16b+12+w = edst w (replicated all 8 blocks)."""
    F = Wa.shape[0]
    nf = Wa.shape[1] - 2 * nh
    wp = np.zeros((F, 128), np.float32)
    for f in range(nf):
        b, u = f // 8, f % 8
        wp[:, 16 * b + u] = Wa[:, 2 * nh + f]
    for v in range(nh):
        wp[:, 16 * v + 8] = Wa[:, nh + v]
    for b in range(8):
        for w in range(nh):
            wp[:, 16 * b + 12 + w] = Wa[:, w]
    return wp


# ----------------------------------------------------------------------------
# Bass program
# ----------------------------------------------------------------------------

def build_program(node_pad, infl, n_batches, n_devices=8, mock_collective=False,
                  stop_after=None, debug_dump=False):
    nc = bacc.Bacc("TRN2", target_bir_lowering=False, debug=False,
                   num_devices=n_devices)
    NB = n_batches
    E_BLK = PAIRS_PER_BATCH * P          # 4096 src slots per side per batch
    IDXC = E_BLK // 16                   # 256 idx cols

    ins = {}

    def inp(name, shape, dtype=F32):
        ins[name] = nc.dram_tensor(name, list(shape), dtype,
                                   kind="ExternalInput")
        return ins[name]

    xT = inp("xT", [P, 2 * node_pad])
    W1p = inp("W1p", [F_IN, 128])
    W2p = inp("W2p", [HC, 128])
    W3p = inp("W3p", [HC, 128])
    b12T = [inp("b1T", [HC, 1]), inp("b2T", [HC, 1])]
    b3T = inp("b3T", [N_CLS, 1])
    E4p = inp("E4p", [HC + H, HC])       # rows 64..67 = head indicator
    E1p = inp("E1p", [2 * N_CLS + 1, N_CLS])
    iota = inp("iota", [P, 16])
    maskT = inp("maskT", [P, 256], mybir.dt.uint8)  # 1 if own half == 1
    for t in ("1", "2"):
        inp(f"srcA{t}", [NB, P, IDXC], I16)
        inp(f"srcB{t}", [NB, P, IDXC], I16)
        inp(f"dstP{t}", [NB, P, 64], I16)
    inp("slotA", [NB, P, PAIRS_PER_BATCH])
    inp("slotB", [NB, P, PAIRS_PER_BATCH])
    outT = nc.dram_tensor("outT", [N_CLS, infl], F32, kind="ExternalOutput")
    dbg = {}
    if debug_dump:
        for nm in ("dbg_xn0", "dbg_xn1"):
            dbg[nm] = nc.dram_tensor(nm, [HC, infl], F32, kind="ExternalOutput")

    GROUPS = [[2 * b_ + 0, 2 * b_ + 1] for b_ in range(n_devices // 2)]

    with tile.TileContext(nc) as tc:
        with (
            tc.tile_pool(name="dram", bufs=1, space="DRAM") as dp,
            tc.tile_pool(name="tbl", bufs=1) as tp,
            tc.tile_pool(name="const", bufs=1) as cp,
            tc.tile_pool(name="mm", bufs=3) as mp,
            tc.tile_pool(name="edge", bufs=2) as ep,
            tc.tile_pool(name="norm", bufs=2) as np_,
            tc.tile_pool(name="psm", bufs=2, space="PSUM") as ps_m,
            tc.tile_pool(name="pse", bufs=2, space="PSUM") as ps_e,
            tc.tile_pool(name="psx", bufs=2, space="PSUM") as ps_x,
        ):
            xn = [dp.tile([HC, infl], F32, tag=f"xn{l}", name=f"xn{l}") for l in range(2)]
            xnf = [dp.tile([2 * HC, infl], F32, tag=f"xnf{l}", name=f"xnf{l}") for l in range(2)]

            # persistent SBUF table, both planes packed as int32 words
            I32 = mybir.dt.int32
            T = tp.tile([P, infl], I32, name="T")

            def t_plane(cols, m):
                return T[:].bitcast(BF16).rearrange(
                    "p (n j) -> p n j", j=2)[:, 0:cols, m : m + 1]

            w1_t = cp.tile([F_IN, 128], F32)
            w2_t = cp.tile([HC, 128], F32)
            w3_t = cp.tile([HC, 128], F32)
            b1_t = cp.tile([HC, 1], F32)
            b2_t = cp.tile([HC, 1], F32)
            b3_t = cp.tile([N_CLS, 1], F32)
            e4_t = cp.tile([HC + H, HC], F32)
            e1_t = cp.tile([2 * N_CLS + 1, N_CLS], F32)
            io_t = cp.tile([P, 16], F32)
            mk_t = cp.tile([P, 256], mybir.dt.uint8)
            for t_, d_ in [(w1_t, W1p), (w2_t, W2p), (w3_t, W3p),
                           (b1_t, ins["b1T"]), (b2_t, ins["b2T"]),
                           (b3_t, b3T), (e4_t, E4p), (e1_t, E1p),
                           (io_t, iota), (mk_t, maskT)]:
                nc.sync.dma_start(out=t_[:], in_=d_[:, :])

            def build_tables(w_t, src_ap, kdim, ncols):
                """Fill T[:, 0:ncols, m] for both planes via PE matmuls.

                src_ap: f(m) -> DRAM AP [kdim, ncols]."""
                nch = ncols // 512
                for m in range(2):
                    sap = src_ap(m)
                    for q in range(nch):
                        xc = mp.tile([kdim, 512], F32, tag="xc")
                        nc.sync.dma_start(out=xc[:],
                                          in_=sap[:, q * 512 : (q + 1) * 512])
                        psm = ps_m.tile([P, 512], F32, space="PSUM", tag="psm")
                        nc.tensor.matmul(out=psm[:], lhsT=w_t[:], rhs=xc[:],
                                         start=True, stop=True)
                        nc.vector.tensor_copy(
                            out=t_plane(ncols, m)[
                                :, q * 512 : (q + 1) * 512, :].rearrange(
                                "p n j -> p (n j)"),
                            in_=psm[:])

            def edge_phase(layer, ne, idx_tag, nh, bias_t, exp_t, relu,
                           out_cols, out_dst):
                nblk_h = (HC if nh == H else N_CLS) // 8  # 8 or 2
                if layer < 3:
                    RW = HC + H            # [s*h 64 | s 4]
                    dn0, dn1 = HC, RW
                else:
                    RW = 2 * N_CLS + 1     # [s*h 16 | zeros 16 | s 1]
                    dn0, dn1 = 2 * N_CLS, RW
                for k in range(NB):
                    sA = ep.tile([P, IDXC], I16, tag="sA")
                    sB = ep.tile([P, IDXC], I16, tag="sB")
                    sD = ep.tile([P, 64], I16, tag="sD")
                    slA = ep.tile([P, PAIRS_PER_BATCH], F32, tag="slA")
                    slB = ep.tile([P, PAIRS_PER_BATCH], F32, tag="slB")
                    nc.sync.dma_start(out=sA[:], in_=ins[f"srcA{idx_tag}"][k])
                    nc.sync.dma_start(out=sB[:], in_=ins[f"srcB{idx_tag}"][k])
                    nc.sync.dma_start(out=sD[:], in_=ins[f"dstP{idx_tag}"][k])
                    nc.sync.dma_start(out=slA[:], in_=ins["slotA"][k])
                    nc.sync.dma_start(out=slB[:], in_=ins["slotB"][k])
                    # dst gather (pair-bucketed, per-core idx) + realign
                    gD = ep.tile([P, 1024], mybir.dt.int32, tag="gD")
                    nc.gpsimd.ap_gather(
                        out_ap=gD[:], in_ap=T[:][:, 0:ne], idxs_ap=sD[:],
                        channels=P, num_elems=ne, d=1, num_idxs=1024)
                    Rd = []
                    for j in range(2):
                        cpd = ep.tile([P, 1024], BF16, tag=f"cpd{j}")
                        nc.vector.tensor_copy(
                            out=cpd[:],
                            in_=gD[:].bitcast(BF16).rearrange(
                                "p (n j) -> p n j", j=2)[
                                :, :, j : j + 1].rearrange("p n j -> p (n j)"))
                        rd = ep.tile([P, 8, P], BF16, tag=f"Rd{j}")
                        nc.sync.dma_start_transpose(out=rd[:], in_=cpd[:])
                        Rd.append(rd)
                    if EDGE_DEPTH == 1:
                        continue
                    # own-half edst select: Et[p, cc, g, w]
                    Et = ep.tile([P, 8, 8, nh], F32, tag="Et")
                    E1_ = ep.tile([P, 8, 8, nh], F32, tag="E1_")
                    dsel = [rd[:].rearrange("p c (g r) -> p c g r", r=16)[
                        :, :, :, 12 : 12 + nh] for rd in Rd]
                    nc.vector.tensor_copy(out=Et[:], in_=dsel[0])
                    nc.vector.tensor_copy(out=E1_[:], in_=dsel[1])
                    nc.vector.copy_predicated(
                        out=Et[:],
                        mask=mk_t[:].rearrange("p (c g w) -> p c g w", c=8,
                                               g=8)[:, :, :, 0:nh],
                        data=E1_[:])
                    for hemi in range(2):
                        RsH = []
                        for s_i, sidx in enumerate((sA, sB)):
                            gS = ep.tile([P, 2048], mybir.dt.int32,
                                          tag="gS")
                            nc.gpsimd.ap_gather(
                                out_ap=gS[:], in_ap=T[:][:, 0:ne],
                                idxs_ap=sidx[:, 128 * hemi : 128 * (hemi + 1)],
                                channels=P, num_elems=ne, d=1, num_idxs=2048)
                            cps = ep.tile([P, 2048], BF16, tag="cps")
                            nc.scalar.activation(
                                out=cps[:],
                                in_=gS[:].bitcast(BF16).rearrange(
                                    "p (n j) -> p n j", j=2)[
                                    :, :, s_i : s_i + 1].rearrange(
                                    "p n j -> p (n j)"),
                                func=mybir.ActivationFunctionType.Identity)
                            rs_t = ep.tile([P, 16, P], BF16, tag=f"RsH{s_i}")
                            nc.sync.dma_start_transpose(out=rs_t[:],
                                                        in_=cps[:])
                            RsH.append(rs_t)
                        if EDGE_DEPTH == 2:
                            continue
                        for st2 in range(2):
                            st = 2 * hemi + st2
                            rs_s = {}
                            it_s = {}
                            for s_i, slS in enumerate((slA, slB)):
                                rsv = RsH[s_i][:][
                                    :, 8 * st2 : 8 * (st2 + 1), :].rearrange(
                                    "p c (b r) -> p c b r", r=16)
                                z = np_.tile([P, 8 * nh], F32, tag="z")
                                zv = z[:].rearrange("p (c h) -> p c h", h=nh)
                                nc.vector.tensor_tensor(
                                    out=zv,
                                    in0=rsv[:, :, 0:nh, 8:9].rearrange(
                                        "p c b r -> p c (b r)"),
                                    in1=Et[:][:, 2 * st + s_i :
                                              2 * st + s_i + 1, :, :].rearrange(
                                        "p c g w -> p (c g) w"),
                                    op=mybir.AluOpType.add)
                                nc.vector.scalar_tensor_tensor(
                                    out=z[:], in0=z[:], scalar=NEG_SLOPE,
                                    in1=z[:], op0=mybir.AluOpType.mult,
                                    op1=mybir.AluOpType.max)
                                s_t = np_.tile([P, 8 * nh], F32, tag="s_t")
                                nc.scalar.activation(
                                    out=s_t[:], in_=z[:],
                                    func=mybir.ActivationFunctionType.Exp)
                                sv = s_t[:].rearrange("p (c h) -> p c h",
                                                      h=nh)
                                r = ep.tile([P, 8 * RW], BF16, tag=f"r{s_i}")
                                rv = r[:].rearrange("p (c w) -> p c w", w=RW)
                                nc.vector.tensor_tensor(
                                    out=rv[:, :, 0 : nblk_h * 8].rearrange(
                                        "p c (h b u) -> p c h b u", b=2, u=8),
                                    in0=rsv[:, :, 0:nblk_h, 0:8].rearrange(
                                        "p c (h b) u -> p c h b u", b=2),
                                    in1=sv.unsqueeze(3).unsqueeze(
                                        4).broadcast_to([P, 8, nh, 2, 8]),
                                    op=mybir.AluOpType.mult)
                                if layer == 3:
                                    nc.vector.memset(
                                        rv[:, :, N_CLS : 2 * N_CLS], 0.0)
                                nc.vector.tensor_copy(out=rv[:, :, dn0:dn1],
                                                      in_=sv)
                                it = ep.tile([P, 8 * 16], BF16, tag=f"i{s_i}")
                                nc.vector.tensor_tensor(
                                    out=it[:].rearrange("p (c i) -> p c i",
                                                        i=16),
                                    in0=slS[:, 8 * st : 8 * (st + 1)].unsqueeze(
                                        2).broadcast_to([P, 8, 16]),
                                    in1=io_t[:].unsqueeze(1).broadcast_to(
                                        [P, 8, 16]),
                                    op=mybir.AluOpType.is_equal)
                                rs_s[s_i] = r
                                it_s[s_i] = it
                            acc = ps_e.tile([RW, P], F32, space="PSUM",
                                            tag="acc")
                            for g in range(8):
                                nc.tensor.matmul(
                                    out=acc[:, 16 * g : 16 * (g + 1)],
                                    lhsT=rs_s[0][:, RW * g : RW * (g + 1)],
                                    rhs=it_s[0][:, 16 * g : 16 * (g + 1)],
                                    start=True, stop=False)
                                nc.tensor.matmul(
                                    out=acc[:, 16 * g : 16 * (g + 1)],
                                    lhsT=rs_s[1][:, RW * g : RW * (g + 1)],
                                    rhs=it_s[1][:, 16 * g : 16 * (g + 1)],
                                    start=False, stop=True)
                            if EDGE_DEPTH == 3:
                                continue
                            seg = np_.tile([dn1, P], F32, tag="seg")
                            nc.vector.tensor_copy(out=seg[:], in_=acc[:])
                            nc.vector.tensor_scalar_add(
                                out=seg[dn0 : dn0 + nh, :],
                                in0=seg[dn0 : dn0 + nh, :], scalar1=EPS)
                            rec = np_.tile([dn1, P], F32, tag="rec")
                            nc.vector.reciprocal(out=rec[dn0 : dn0 + nh, :],
                                                 in_=seg[dn0 : dn0 + nh, :])
                            rxp = ps_x.tile([out_cols, P], F32, space="PSUM",
                                            tag="rxp")
                            nc.tensor.matmul(out=rxp[:],
                                             lhsT=exp_t[dn0 : dn0 + nh, :],
                                             rhs=rec[dn0 : dn0 + nh, :],
                                             start=True, stop=True)
                            rex = np_.tile([out_cols, P], F32, tag="rex")
                            nc.vector.tensor_copy(out=rex[:], in_=rxp[:])
                            o_t = np_.tile([out_cols, P], F32, tag="o")
                            nc.vector.tensor_tensor(out=o_t[:],
                                                    in0=seg[:out_cols, :],
                                                    in1=rex[:],
                                                    op=mybir.AluOpType.mult)
                            nc.scalar.activation(
                                out=o_t[:], in_=o_t[:],
                                func=(mybir.ActivationFunctionType.Relu
                                      if relu else
                                      mybir.ActivationFunctionType.Identity),
                                bias=bias_t[:, :1], scale=1.0)
                            col0 = (k * 4 + st) * P
                            nc.sync.dma_start(
                                out=out_dst[:, col0 : col0 + P], in_=o_t[:])

            stop = [False]

            def _chk(tag):
                if stop[0] or stop_after == tag:
                    stop[0] = True
                return stop[0]

            # ---------------- layer 1 ----------------
            build_tables(w1_t, lambda m: xT[:, m * node_pad : (m + 1) * node_pad],
                         F_IN, node_pad)
            if not _chk("m1"):
                edge_phase(1, node_pad, "1", H, b1_t, e4_t, True, HC,
                           xn[0][:])
            if debug_dump and not stop[0]:
                nc.sync.dma_start(out=dbg["dbg_xn0"][:, :], in_=xn[0][:][:, :])
            if not _chk("e1"):
                if mock_collective:
                    nc.sync.dma_start(out=xnf[0][:][:HC, :], in_=xn[0][:][:, :])
                    nc.sync.dma_start(out=xnf[0][:][HC:, :], in_=xn[0][:][:, :])
                else:
                    nc.gpsimd.collective_compute(
                        "AllGather", mybir.AluOpType.bypass,
                        replica_groups=GROUPS,
                        ins=[xn[0][:][:, :]], outs=[xnf[0][:][:, :]])
            # ---------------- layer 2 ----------------
            xnf0 = xnf[0][:]
            if not _chk("x1"):
                build_tables(w2_t, lambda m: xnf0[m * HC : (m + 1) * HC, :],
                             HC, infl)
            if not _chk("m2"):
                edge_phase(2, infl, "2", H, b2_t, e4_t, True, HC, xn[1][:])
            if debug_dump and not stop[0]:
                nc.sync.dma_start(out=dbg["dbg_xn1"][:, :], in_=xn[1][:][:, :])
            if not _chk("e2"):
                if mock_collective:
                    nc.sync.dma_start(out=xnf[1][:][:HC, :], in_=xn[1][:][:, :])
                    nc.sync.dma_start(out=xnf[1][:][HC:, :], in_=xn[1][:][:, :])
                else:
                    nc.gpsimd.collective_compute(
                        "AllGather", mybir.AluOpType.bypass,
                        replica_groups=GROUPS,
                        ins=[xn[1][:][:, :]], outs=[xnf[1][:][:, :]])
            # ---------------- layer 3 ----------------
            xnf1 = xnf[1][:]
            if not _chk("x2"):
                build_tables(w3_t, lambda m: xnf1[m * HC : (m + 1) * HC, :],
                             HC, infl)
            if not _chk("m3"):
                edge_phase(3, infl, "2", 1, b3_t, e1_t, False, N_CLS,
                           outT[:, :])

    nc.compile()
    return nc


# ----------------------------------------------------------------------------
# Entry point
# ----------------------------------------------------------------------------

BUILD_KWARGS = {}


def kernel(**inputs):
    x = np.asarray(inputs["x"], np.float32)
    edge_index = np.asarray(inputs["edge_index"])
    Bc, Nn, Fi = x.shape
    pp = preprocess(edge_index, Nn)
    half, node_pad, infl = pp["half"], pp["node_pad"], pp["infl"]

    W1p = pack_weights(augment_weights(inputs["W1"], inputs["a1s"], inputs["a1d"]), H)
    W2p = pack_weights(augment_weights(inputs["W2"], inputs["a2s"], inputs["a2d"]), H)
    W3p = pack_weights(augment_weights(inputs["W3"], inputs["a3s"], inputs["a3d"]), 1)
    b1 = np.asarray(inputs["b1"], np.float32).reshape(HC, 1)
    b2 = np.asarray(inputs["b2"], np.float32).reshape(HC, 1)
    b3 = np.asarray(inputs["b3"], np.float32).reshape(N_CLS, 1)
    E4p = np.zeros((HC + H, HC), np.float32)
    for hh in range(H):
        E4p[HC + hh, hh * C : (hh + 1) * C] = 1.0
    E1p = np.zeros((2 * N_CLS + 1, N_CLS), np.float32)
    E1p[2 * N_CLS, :] = 1.0
    iota = np.broadcast_to(np.arange(16, dtype=np.float32), (P, 16)).copy()

    nc = build_program(node_pad, infl, pp["n_batches"], n_devices=8,
                       **BUILD_KWARGS)

    idx_half = [build_core_idx_arrays(pp, h) for h in range(2)]
    in_maps = []
    for c in range(8):
        b_, h_ = c // 2, c % 2
        xTb = np.zeros((P, 2 * node_pad), np.float32)
        xTb[:, :half] = x[b_, :half].T
        xTb[:, node_pad : node_pad + half] = x[b_, half:].T
        m = dict(
            xT=xTb,
            W1p=W1p, W2p=W2p, W3p=W3p, b1T=b1, b2T=b2, b3T=b3,
            E4p=E4p, E1p=E1p, iota=iota,
            maskT=np.full((P, 256), h_, np.uint8),
        )
        m.update(idx_half[h_])
        in_maps.append(m)

    res = run_bass_kernel_spmd(nc, in_maps, core_ids=list(range(8)))

    out = np.zeros((Bc, Nn, N_CLS), np.float32)
    for c in range(8):
        b_, h_ = c // 2, c % 2
        o = res.results[c]["outT"]  # [N_CLS, infl]
        inv = pp["packs"][h_]["infl_of_node"]
        out[b_, h_ * half : (h_ + 1) * half] = o[:, inv].T
    return out
